# revision 1
# baseline (speedup 1.0000x reference)
"""GCN (2-layer) Trainium2 kernel, 8-core SPMD — v3b.

vs v2: 128-dst windows (halves PE matmul width and DVE one-hot elements; L2
padding is dominated by cross-core variance so tile counts are unchanged),
one batched tensor_tensor is_equal per group builds all its one-hot tiles,
L2 self-contributions read h windows retained in SBUF from layer 1 (no DMA
round-trip), output windows are staged per 14-window batch and written with
one DMA, and the layer-2 dma_gather calls round-robin 4 SWDGE queues.
"""

import sys

sys.path.insert(0, "/opt/trn_rl_repo")

import numpy as np
import ml_dtypes

import concourse.mybir as mybir
from concourse import bacc
from concourse.tile import TileContext

N = 100000
E = 600000
IN_D = 128
HID_D = 128
OUT_D = 64
NCORES = 8
BLK = 128
NW = 98
SH = BLK * NW            # 12544
NPADN = NCORES * SH      # 100352
HS = SH // 2
CHUNK = NPADN // 4       # 25088
NCHUNK = 4
BATCH_W = 7              # windows per gather call / output batch
NBATCH = NW // BATCH_W   # 14
SENT = 300.0

f32 = mybir.dt.float32
f16 = mybir.dt.float16
bf16 = mybir.dt.bfloat16
i16 = mybir.dt.int16
bf16np = ml_dtypes.bfloat16

_cache = {}


def _posmap():
    g = np.arange(NPADN)
    r, row = g // SH, g % SH
    return np.where(row < HS, r * HS + row, NCORES * HS + r * HS + (row - HS))


def _preprocess(edge_index):
    src_e = edge_index[0]
    dst_e = edge_index[1]
    deg = np.bincount(np.concatenate([dst_e, np.arange(N)]), minlength=N).astype(np.float32)
    dinv = 1.0 / np.sqrt(deg)
    sdeg = np.sqrt(deg)

    pos_g = _posmap()
    pos_of = np.zeros(NPADN, np.int64)
    pos_of[:N] = pos_g[:N]
    core = dst_e // SH

    # ---- Layer 1: host stream, groups = windows (edges + self-loops) ----
    percore_l1 = []
    for c in range(NCORES):
        m = core == c
        s, d = src_e[m], dst_e[m]
        selfs = np.arange(c * SH, min((c + 1) * SH, N))
        s = np.concatenate([s, selfs])
        d = np.concatenate([d, selfs])
        w = (d % SH) // BLK
        order = np.argsort(w, kind="stable")
        percore_l1.append((s[order], d[order], w[order]))
    cnt1 = np.zeros((NCORES, NW), np.int64)
    for c in range(NCORES):
        cnt1[c] = np.bincount(percore_l1[c][2], minlength=NW)
    T1_tiles = np.maximum((cnt1.max(axis=0) + BLK - 1) // BLK, 1)
    T1_off = np.concatenate([[0], np.cumsum(T1_tiles)])
    T1 = int(T1_off[-1])
    l1_src, l1_dstrel = [], []
    for c in range(NCORES):
        s, d, w = percore_l1[c]
        srcs = np.full(T1 * BLK, -1, np.int64)
        drel = np.full(T1 * BLK, SENT, np.float32)
        gstart = np.concatenate([[0], np.cumsum(cnt1[c])])
        within = np.arange(len(s)) - gstart[w]
        slot = T1_off[w] * BLK + within
        srcs[slot] = s
        drel[slot] = d % BLK
        l1_src.append(srcs)
        l1_dstrel.append(drel.reshape(T1, BLK).T.astype(np.float16))

    # ---- Layer 2: groups (window, chunk), exact-packed ----
    percore_l2 = []
    for c in range(NCORES):
        m = core == c
        s, d = src_e[m], dst_e[m]
        p = pos_of[s]
        k = p // CHUNK
        w = (d % SH) // BLK
        g = w * NCHUNK + k
        order = np.argsort(g, kind="stable")
        percore_l2.append((p[order] % CHUNK, d[order], g[order]))
    cnt2 = np.zeros((NCORES, NW * NCHUNK), np.int64)
    for c in range(NCORES):
        cnt2[c] = np.bincount(percore_l2[c][2], minlength=NW * NCHUNK)
    T2_tiles = np.maximum((cnt2.max(axis=0) + BLK - 1) // BLK, 1)

    call_list = []  # (passid, b, k, [group ids], ntiles)
    for passid, ks in ((0, (0, 1)), (1, (2, 3))):
        for b in range(NBATCH):
            for k in ks:
                gs = [(b * BATCH_W + i) * NCHUNK + k for i in range(BATCH_W)]
                call_list.append((passid, b, k, gs, int(T2_tiles[gs].sum())))
    tile_off2 = np.zeros(NW * NCHUNK, np.int64)
    call_toff = []
    t = 0
    for (_, _, _, gs, nt) in call_list:
        call_toff.append(t)
        for g in gs:
            tile_off2[g] = t
            t += int(T2_tiles[g])
    T2 = t
    L2 = T2 * BLK

    idx_arrs = np.zeros((NCORES, 128, L2 // 16), np.int16)
    dstrel2 = np.full((NCORES, 128, T2), SENT, np.float16)
    for c in range(NCORES):
        rel, d, g = percore_l2[c]
        gstart = np.concatenate([[0], np.cumsum(cnt2[c])])
        within = np.arange(len(g)) - gstart[g]
        slot = tile_off2[g] * BLK + within
        idxw = np.zeros(L2, np.int16)
        idxw[slot] = rel.astype(np.int16)
        idx_arrs[c] = np.tile(idxw.reshape(L2 // 16, 16).T, (8, 1))
        drel = np.full(L2, SENT, np.float32)
        drel[slot] = d % BLK
        dstrel2[c] = drel.reshape(T2, BLK).T.astype(np.float16)

    return dict(
        deg=deg, dinv=dinv, sdeg=sdeg,
        T1_tiles=T1_tiles, T1_off=T1_off, T1=T1,
        l1_src=l1_src, l1_dstrel=l1_dstrel,
        T2_tiles=T2_tiles, T2=T2, L2=L2,
        call_list=call_list, call_toff=call_toff, tile_off2=tile_off2,
        idx_arrs=idx_arrs, dstrel2=dstrel2,
    )


def _build_nc(meta):
    T1, T2, L2 = meta["T1"], meta["T2"], meta["L2"]
    T1_tiles, T1_off = meta["T1_tiles"], meta["T1_off"]
    T2_tiles = meta["T2_tiles"]
    call_list, call_toff = meta["call_list"], meta["call_toff"]
    tile_off2 = meta["tile_off2"]

    nc = bacc.Bacc(None, target_bir_lowering=False,
                   dynamic_dma_scratch_size=65536, num_swdge_queues=4)

    stream_d = nc.dram_tensor("stream", [128, T1 * IN_D], bf16, kind="ExternalInput")
    dstrel1_d = nc.dram_tensor("dstrel1", [128, T1], f16, kind="ExternalInput")
    idx2_d = nc.dram_tensor("idx2", [128, L2 // 16], i16, kind="ExternalInput")
    dstrel2_d = nc.dram_tensor("dstrel2", [128, T2], f16, kind="ExternalInput")
    iota_d = nc.dram_tensor("iota", [128, BLK], f16, kind="ExternalInput")
    pidx_d = nc.dram_tensor("pidx", [128, 1], f32, kind="ExternalInput")
    w1_d = nc.dram_tensor("w1", [IN_D, HID_D], bf16, kind="ExternalInput")
    w2_d = nc.dram_tensor("w2", [HID_D, OUT_D], bf16, kind="ExternalInput")
    b1_d = nc.dram_tensor("b1", [1, HID_D], f32, kind="ExternalInput")
    b2_d = nc.dram_tensor("b2", [1, OUT_D], f32, kind="ExternalInput")
    dinv2_d = nc.dram_tensor("dinv2", [128, NW], f32, kind="ExternalInput")
    dinv_d = nc.dram_tensor("dinv", [128, NW], f32, kind="ExternalInput")
    sdeg_d = nc.dram_tensor("sdeg", [1, SH], f32, kind="ExternalInput")
    out_d = nc.dram_tensor("out", [SH, OUT_D], f32, kind="ExternalOutput")

    h_send_a = nc.dram_tensor("h_send_a", [HS, HID_D], bf16)
    h_send_b = nc.dram_tensor("h_send_b", [HS, HID_D], bf16)
    h_full_a = nc.dram_tensor("h_full_a", [NCORES * HS, HID_D], bf16, addr_space="Shared")
    h_full_b = nc.dram_tensor("h_full_b", [NCORES * HS, HID_D], bf16, addr_space="Shared")

    def h_send_ap(w):
        half, wr = w // (NW // 2), w % (NW // 2)
        hd = h_send_a if half == 0 else h_send_b
        return hd[wr * BLK:(wr + 1) * BLK, :]

    with TileContext(nc) as tc:
        with (
            tc.tile_pool(name="const", bufs=1) as constp,
            tc.tile_pool(name="stream", bufs=3) as streamp,
            tc.tile_pool(name="gath", bufs=3) as gathp,
            tc.tile_pool(name="sbuild", bufs=3) as sp,
            tc.tile_pool(name="agg", bufs=3) as aggp,
            tc.tile_pool(name="outp", bufs=2) as outp,
            tc.tile_pool(name="sdg", bufs=2) as sdgp,
            tc.tile_pool(name="psum_seg", bufs=3, space="PSUM") as psegp,
            tc.tile_pool(name="psum_h", bufs=2, space="PSUM") as phk,
        ):
            dstrel1_t = constp.tile([128, T1], f16, tag="dstrel1")
            idx2_t = constp.tile([128, L2 // 16], i16, tag="idx2")
            dstrel2_t = constp.tile([128, T2], f16, tag="dstrel2")
            iota_t = constp.tile([128, BLK], f16, tag="iota")
            pidx_t = constp.tile([128, 1], f32, tag="pidx")
            w1_t = constp.tile([IN_D, HID_D], bf16, tag="w1")
            w2_t = constp.tile([HID_D, OUT_D], bf16, tag="w2")
            b1_t = constp.tile([1, HID_D], f32, tag="b1")
            b2_t = constp.tile([1, OUT_D], f32, tag="b2")
            dinv2_t = constp.tile([128, NW], f32, tag="dinv2")
            dinv_t = constp.tile([128, NW], f32, tag="dinv")
            s_id = constp.tile([128, BLK], bf16, tag="s_id")
            nc.sync.dma_start(out=dstrel1_t[:], in_=dstrel1_d[:])
            nc.sync.dma_start(out=idx2_t[:], in_=idx2_d[:])
            nc.sync.dma_start(out=dstrel2_t[:], in_=dstrel2_d[:])
            nc.sync.dma_start(out=iota_t[:], in_=iota_d[:])
            nc.sync.dma_start(out=pidx_t[:], in_=pidx_d[:])
            nc.sync.dma_start(out=w1_t[:], in_=w1_d[:])
            nc.sync.dma_start(out=w2_t[:], in_=w2_d[:])
            nc.sync.dma_start(out=b1_t[:], in_=b1_d[:])
            nc.sync.dma_start(out=b2_t[:], in_=b2_d[:])
            nc.sync.dma_start(out=dinv2_t[:], in_=dinv2_d[:])
            nc.sync.dma_start(out=dinv_t[:], in_=dinv_d[:])
            nc.vector.tensor_scalar(
                s_id[:], iota_t[:], pidx_t[:, 0:1], None,
                mybir.AluOpType.is_equal)

            def sbuild_batch(dstrel_t, t0, nt):
                """One is_equal for nt one-hot tiles [128, nt, 128]."""
                s = sp.tile([128, nt, BLK], bf16, tag="s", name=f"s{t0}")
                a = iota_t[:].unsqueeze(1).broadcast_to([128, nt, BLK])
                b = dstrel_t[:, t0:t0 + nt].unsqueeze(2).broadcast_to([128, nt, BLK])
                nc.vector.tensor_tensor(s[:], a, b, mybir.AluOpType.is_equal)
                return s

            # ------------------------- Layer 1 -------------------------
            hbatch = []
            for b in range(NBATCH):
                sdg = sdgp.tile([1, BATCH_W * BLK], f32, tag="sdg")
                nc.sync.dma_start(
                    out=sdg[:],
                    in_=sdeg_d[0:1, b * BATCH_W * BLK:(b + 1) * BATCH_W * BLK])
                ob = constp.tile([128, BATCH_W, HID_D], bf16, tag=f"hb{b}",
                                 name=f"hb{b}")
                hbatch.append(ob)
                for i in range(BATCH_W):
                    w = b * BATCH_W + i
                    nt = int(T1_tiles[w])
                    t0 = int(T1_off[w])
                    m = streamp.tile([128, nt, IN_D], bf16, tag="m1")
                    nc.scalar.dma_start(out=m[:], in_=stream_d[:, t0 * IN_D:(t0 + nt) * IN_D])
                    s = sbuild_batch(dstrel1_t, t0, nt)
                    pseg = psegp.tile([128, BLK], f32, tag="pseg")
                    for j in range(nt):
                        nc.tensor.matmul(pseg[:], m[:, j, :], s[:, j, :],
                                         start=(j == 0), stop=(j == nt - 1))
                    aggT = aggp.tile([128, BLK], bf16, tag="agg")
                    nc.scalar.activation(aggT[:], pseg[:],
                                         mybir.ActivationFunctionType.Copy)
                    ph = phk.tile([128, HID_D], f32, tag="ph")
                    nc.tensor.matmul(ph[:], aggT[:], w1_t[:], start=True, stop=False)
                    nc.tensor.matmul(ph[:], sdg[0:1, i * BLK:(i + 1) * BLK],
                                     b1_t[:], start=False, stop=True)
                    nc.scalar.activation(ob[:, i, :], ph[:],
                                         mybir.ActivationFunctionType.Relu,
                                         scale=dinv2_t[:, w:w + 1])
                for i in range(BATCH_W):
                    w = b * BATCH_W + i
                    nc.sync.dma_start(out=h_send_ap(w), in_=ob[:, i, :])

            # ---------------------- h exchange ----------------------
            nc.gpsimd.collective_compute(
                "AllGather", mybir.AluOpType.bypass,
                replica_groups=[list(range(NCORES))],
                ins=[h_send_a[:]], outs=[h_full_a[:]])
            nc.gpsimd.collective_compute(
                "AllGather", mybir.AluOpType.bypass,
                replica_groups=[list(range(NCORES))],
                ins=[h_send_b[:]], outs=[h_full_b[:]])
            h_tables = [
                h_full_a[0:CHUNK, :], h_full_a[CHUNK:2 * CHUNK, :],
                h_full_b[0:CHUNK, :], h_full_b[CHUNK:2 * CHUNK, :],
            ]

            # ------------------------- Layer 2 -------------------------
            aggA = {}
            qn = [0]

            def gather_call(ci):
                (_, b, k, gs, nt) = call_list[ci]
                t0 = call_toff[ci]
                g = gathp.tile([128, nt, HID_D], bf16, tag=f"g{k}",
                               name=f"g{ci}")
                nidx = nt * BLK
                nc.gpsimd.dma_gather(
                    g[:], h_tables[k],
                    idx2_t[:, t0 * 8: t0 * 8 + nidx // 16],
                    num_idxs=nidx, num_idxs_reg=nidx, elem_size=HID_D,
                    single_packet=False, queue_num=qn[0] % 4,
                )
                qn[0] += 1
                return g

            ci = 0
            for b in range(NBATCH):
                bufs = {}
                for k in (0, 1):
                    bufs[k] = (gather_call(ci), call_toff[ci])
                    ci += 1
                for i in range(BATCH_W):
                    w = b * BATCH_W + i
                    pseg = psegp.tile([128, BLK], f32, tag="pseg")
                    nc.tensor.matmul(pseg[:], hbatch[w // BATCH_W][:, w % BATCH_W, :],
                                     s_id[:], start=True, stop=False)
                    for k in (0, 1):
                        g, ct0 = bufs[k]
                        gid = w * NCHUNK + k
                        col0 = int(tile_off2[gid] - ct0)
                        ntg = int(T2_tiles[gid])
                        s = sbuild_batch(dstrel2_t, int(tile_off2[gid]), ntg)
                        for j in range(ntg):
                            nc.tensor.matmul(pseg[:], g[:, col0 + j, :], s[:, j, :],
                                             start=False,
                                             stop=(k == 1 and j == ntg - 1))
                    ag = constp.tile([128, BLK], bf16, tag=f"aggA{w}",
                                     name=f"aggA{w}")
                    nc.vector.tensor_copy(ag[:], pseg[:])
                    aggA[w] = ag

            for b in range(NBATCH):
                bufs = {}
                for k in (2, 3):
                    bufs[k] = (gather_call(ci), call_toff[ci])
                    ci += 1
                sdg = sdgp.tile([1, BATCH_W * BLK], f32, tag="sdg")
                nc.sync.dma_start(
                    out=sdg[:],
                    in_=sdeg_d[0:1, b * BATCH_W * BLK:(b + 1) * BATCH_W * BLK])
                ob = outp.tile([128, BATCH_W, OUT_D], f32, tag="ob2")
                for i in range(BATCH_W):
                    w = b * BATCH_W + i
                    pseg = psegp.tile([128, BLK], f32, tag="pseg")
                    first = True
                    for k in (2, 3):
                        g, ct0 = bufs[k]
                        gid = w * NCHUNK + k
                        col0 = int(tile_off2[gid] - ct0)
                        ntg = int(T2_tiles[gid])
                        s = sbuild_batch(dstrel2_t, int(tile_off2[gid]), ntg)
                        for j in range(ntg):
                            nc.tensor.matmul(pseg[:], g[:, col0 + j, :], s[:, j, :],
                                             start=first,
                                             stop=(k == 3 and j == ntg - 1))
                            first = False
                    aggB = aggp.tile([128, BLK], bf16, tag="aggB")
                    nc.scalar.activation(aggB[:], pseg[:],
                                         mybir.ActivationFunctionType.Copy)
                    ph = phk.tile([128, OUT_D], f32, tag="ph2")
                    nc.tensor.matmul(ph[:], aggA[w][:], w2_t[:], start=True, stop=False)
                    nc.tensor.matmul(ph[:], aggB[:], w2_t[:], start=False, stop=False)
                    nc.tensor.matmul(ph[:], sdg[0:1, i * BLK:(i + 1) * BLK],
                                     b2_t[:], start=False, stop=True)
                    nc.scalar.activation(ob[:, i, :], ph[:],
                                         mybir.ActivationFunctionType.Copy,
                                         scale=dinv_t[:, w:w + 1])
                for i in range(BATCH_W):
                    w = b * BATCH_W + i
                    nc.sync.dma_start(out=out_d[w * BLK:(w + 1) * BLK, :],
                                  in_=ob[:, i, :])

    nc.compile()
    return nc


def _get_runner(edge_index_bytes, edge_index):
    key = hash(edge_index_bytes)
    if key in _cache:
        return _cache[key]
    meta = _preprocess(edge_index.astype(np.int64))
    nc = _build_nc(meta)
    runner = _Runner(nc)
    _cache[key] = (meta, nc, runner)
    return _cache[key]


def _in_maps(meta, x, W1, b1, W2, b2):
    dinv = meta["dinv"]
    sdeg = meta["sdeg"]
    T1 = meta["T1"]
    xs = (x * dinv[:, None]).astype(np.float32)
    iota = np.broadcast_to(np.arange(BLK, dtype=np.float16), (128, BLK)).copy()
    dinv_p = np.concatenate([dinv, np.zeros(NPADN - N, np.float32)])
    sdeg_p = np.concatenate([sdeg, np.zeros(NPADN - N, np.float32)])
    maps = []
    for c in range(NCORES):
        srcs = meta["l1_src"][c]
        rows = xs[np.clip(srcs, 0, N - 1)]
        rows[srcs < 0] = 0.0
        stream = np.ascontiguousarray(
            rows.reshape(T1, BLK, IN_D).transpose(1, 0, 2)
        ).reshape(128, T1 * IN_D).astype(bf16np)
        dv = dinv_p[c * SH:(c + 1) * SH].reshape(NW, 128).T.copy()
        maps.append({
            "stream": stream,
            "dstrel1": meta["l1_dstrel"][c],
            "idx2": meta["idx_arrs"][c],
            "dstrel2": meta["dstrel2"][c],
            "iota": iota,
            "pidx": np.arange(128, dtype=np.float32).reshape(128, 1),
            "w1": np.asarray(W1).astype(bf16np),
            "w2": np.asarray(W2).astype(bf16np),
            "b1": np.asarray(b1).reshape(1, HID_D).astype(np.float32),
            "b2": np.asarray(b2).reshape(1, OUT_D).astype(np.float32),
            "dinv2": (dv * dv).copy(),
            "dinv": dv,
            "sdeg": sdeg_p[c * SH:(c + 1) * SH].reshape(1, SH).copy(),
        })
    return maps


class _Runner:
    """Compile-once PJRT executor for a fixed Bass module (8-core SPMD)."""

    def __init__(self, nc):
        import jax
        from jax.sharding import Mesh, PartitionSpec
        from jax.experimental.shard_map import shard_map
        from concourse import bass2jax

        bass2jax.install_neuronx_cc_hook()
        self.nc = nc
        in_names, out_names, out_avals, zero_shapes = [], [], [], []
        pname = nc.partition_id_tensor.name if nc.partition_id_tensor else None
        for alloc in nc.m.functions[0].allocations:
            if not isinstance(alloc, mybir.MemoryLocationSet):
                continue
            name = alloc.memorylocations[0].name
            if alloc.kind == "ExternalInput":
                if name != pname:
                    in_names.append(name)
            elif alloc.kind == "ExternalOutput":
                out_names.append(name)
                shape = tuple(alloc.tensor_shape)
                dtype = mybir.dt.np(alloc.dtype)
                out_avals.append(jax.core.ShapedArray(shape, dtype))
                zero_shapes.append((shape, dtype))
        self.in_names, self.out_names = in_names, out_names
        self.zero_shapes = zero_shapes
        n_params, n_outs = len(in_names), len(out_names)
        all_names = in_names + out_names + ([pname] if pname else [])

        def _body(*args):
            operands = list(args)
            if pname is not None:
                operands.append(bass2jax.partition_id_tensor())
            outs = bass2jax._bass_exec_p.bind(
                *operands,
                out_avals=tuple(out_avals),
                in_names=tuple(all_names),
                out_names=tuple(out_names),
                lowering_input_output_aliases=(),
                sim_require_finite=True,
                sim_require_nnan=True,
                nc=nc,
            )
            return tuple(outs)

        devices = jax.devices()[:NCORES]
        mesh = Mesh(np.asarray(devices), ("core",))
        self.mesh = mesh
        in_specs = (PartitionSpec("core"),) * (n_params + n_outs)
        out_specs = (PartitionSpec("core"),) * n_outs
        self.fn = jax.jit(
            shard_map(_body, mesh=mesh, in_specs=in_specs, out_specs=out_specs,
                      check_rep=False),
            donate_argnums=tuple(range(n_params, n_params + n_outs)),
            keep_unused=True,
        )
        self.out_avals = out_avals

    def prep(self, in_maps):
        return [
            np.concatenate([np.asarray(in_maps[c][n]) for c in range(NCORES)],
                           axis=0)
            for n in self.in_names
        ]

    def zeros(self):
        return [np.zeros((NCORES * s[0], *s[1:]), d) for s, d in self.zero_shapes]

    def run_raw(self, concat_in, concat_zeros):
        import jax
        out_arrs = self.fn(*concat_in, *concat_zeros)
        jax.block_until_ready(out_arrs)
        return out_arrs

    def __call__(self, concat_in, concat_zeros):
        out_arrs = self.run_raw(concat_in, concat_zeros)
        return {
            n: np.asarray(out_arrs[i]).reshape(
                NCORES, *self.out_avals[i].shape)
            for i, n in enumerate(self.out_names)
        }


def kernel(x, edge_index, W1, b1, W2, b2):
    x = np.asarray(x, np.float32)
    edge_index = np.asarray(edge_index)
    meta, nc, runner = _get_runner(edge_index.tobytes(), edge_index)
    maps = _in_maps(meta, x, np.asarray(W1), np.asarray(b1), np.asarray(W2),
                    np.asarray(b2))
    concat_in = runner.prep(maps)

    def run_once():
        res = runner(concat_in, runner.zeros())
        return res["out"].reshape(NCORES * SH, OUT_D)[:N].astype(np.float32)

    # First post-load execution has (rarely) shown transient corruption;
    # warm up once, then take two runs and require agreement.
    run_once()
    o1 = run_once()
    o2 = run_once()
    if np.array_equal(o1, o2):
        return o1
    for _ in range(3):
        o3 = run_once()
        if np.array_equal(o3, o1):
            return o1
        if np.array_equal(o3, o2):
            return o2
        o1, o2 = o2, o3
    return o2

